# revision 16
# baseline (speedup 1.0000x reference)
"""Trainium2 Bass kernel for nn_AttentionBlock (GroupNorm + MHA + proj + residual).

Full inputs in, full output out. Sharding: 8 cores = 2 batches x 4 query-slices.
Each core: GroupNorm over its batch image (replicated within the batch group),
q projection for its 1024 queries, k/v projections over all 4096 keys,
per-head attention (S^T = k^T q formulation, softmax along the PSUM partition
axis via an appended ones-column in the PV matmul), output projection and
residual for its query slice. Host side only slices/rotates/concatenates.

v2: phase 4 is software-pipelined per head-PAIR: the even head's S groups live
in a 3-bank PSUM pool A, the odd head's in pool B (plus 2 PV banks = 8).
Softmax exp runs as one N=1536 activation per group so ScalarE (the kernel's
throughput floor: ~2 exps of 16K elems per query-window per head) streams with
no gaps; PV matmuls are emitted one period behind S so the tensor engine FIFO
never stalls behind an exp dependency.

All matmuls run in bf16 with fp32 PSUM accumulation; softmax logits stay fp32.
"""
import numpy as np

C = 512          # channels
N = 4096         # pixels (64*64)
NQ = 1024        # queries per core
H = 8            # heads
D = 64           # head dim
T = 4            # 128-channel chunks
W = NQ // 512    # query windows of 512
MT = N // 128    # key m-tiles of 128
NGROUPS = 8
EPS = 1e-5
GELEM = (C // NGROUPS) * N   # elements per norm group
NGRP = 11                    # m-groups per head stream: [3]*10 + [2]

_COMPILED = None


def _emit(tc, io):
    import concourse.bass as bass
    from concourse import mybir
    from contextlib import ExitStack

    nc = tc.nc
    f32 = mybir.dt.float32
    bf16 = mybir.dt.bfloat16
    Alu = mybir.AluOpType
    Act = mybir.ActivationFunctionType

    xb, qkvw, qkvb, projw, projb, nw, nb, y = (
        io["xb"], io["qkvw"], io["qkvb"], io["projw"], io["projb"],
        io["nw"], io["nb"], io["y"])

    ctx = ExitStack()
    with ctx:
        # ---------------- pools ----------------
        # PSUM: pool A (3 banks) = even-head S stream, pool B (3 banks) =
        # odd-head S stream, pv pool 2x1 bank. 3+3+2 = 8 banks. Phase 1/3/5
        # transposes/projection chains borrow A/B between attention uses.
        left = ctx.enter_context(tc.tile_pool(name="left", bufs=1))
        psum_a = ctx.enter_context(tc.tile_pool(name="psum_a", bufs=1, space="PSUM"))
        psum_b = ctx.enter_context(tc.tile_pool(name="psum_b", bufs=1, space="PSUM"))
        psum_pv = ctx.enter_context(tc.tile_pool(name="psum_pv", bufs=2, space="PSUM"))
        pool_ab = [psum_a, psum_b]

        right_ctx = ExitStack()
        xf_pool = right_ctx.enter_context(
            tc.tile_pool(name="xf_pool", bufs=1, side="right"))
        wstg_pool = right_ctx.enter_context(
            tc.tile_pool(name="wstg_pool", bufs=4, side="right"))
        scr_pool = right_ctx.enter_context(
            tc.tile_pool(name="scr_pool", bufs=2, side="right"))

        # ---------------- persistent tiles ----------------
        xn = [left.tile([128, N], bf16, name=f"xn{t}", tag=f"xn{t}") for t in range(T)]
        ksb = [left.tile([128, N], bf16, name=f"ksb{t}", tag=f"ksb{t}") for t in range(T)]
        qsb = [left.tile([128, NQ], bf16, name=f"qsb{t}", tag=f"qsb{t}") for t in range(T)]
        wTq = [left.tile([128, 1536], bf16, name=f"wTq{t}", tag=f"wTq{t}") for t in range(T)]
        wTp = [left.tile([128, C], bf16, name=f"wTp{t}", tag=f"wTp{t}") for t in range(T)]
        vb_bc = left.tile([128, C], f32, name="vb_bc", tag="vb_bc")
        ones_row = left.tile([1, D], f32, name="ones_row", tag="ones_row")
        qb = [left.tile([128, 1], f32, name=f"qb{i}", tag=f"qb{i}") for i in range(8)]
        pb = [left.tile([128, 1], f32, name=f"pb{i}", tag=f"pb{i}") for i in range(T)]
        nwt = [left.tile([128, 1], f32, name=f"nwt{t}", tag=f"nwt{t}") for t in range(T)]
        nbt = [left.tile([128, 1], f32, name=f"nbt{t}", tag=f"nbt{t}") for t in range(T)]
        stat = [left.tile([128, 2], f32, name=f"stat{t}", tag=f"stat{t}") for t in range(T)]
        gstat = [left.tile([128, 2], f32, name=f"gstat{t}", tag=f"gstat{t}") for t in range(T)]

        # ---------------- input DMAs (x tiles 0-1 first) ----------------
        xf = [xf_pool.tile([128, N], f32, name=f"xf{t}", tag=f"xf{t}") for t in range(T)]
        for t in range(2):
            for c4 in range(4):
                nc.sync.dma_start(
                    xf[t][:, 1024 * c4:1024 * (c4 + 1)],
                    xb[128 * t:128 * (t + 1), 1024 * c4:1024 * (c4 + 1)])
        # weights: natural-layout contiguous DMA, cast to bf16, transpose
        # 128x128 blocks on the PE (identity trick) into wTq/wTp.
        ident = left.tile([128, 128], bf16, name="ident", tag="ident")
        nc.sync.dma_start(ident[:], io["cid"][:, :])
        ind = left.tile([128, 2], f32, name="ind", tag="ind")
        nc.sync.dma_start(ind[:], io["cind"][:, :])
        indT = left.tile([2, 128], f32, name="indT", tag="indT")
        nc.sync.dma_start(indT[0:2, :], io["cindT"][:, :])
        for i in range(12):   # qkv_w row-tiles
            wstg = wstg_pool.tile([128, C], f32, name="wstg", tag="wstg")
            nc.sync.dma_start(wstg[:], qkvw[128 * i:128 * (i + 1), :])
            wbf = wstg_pool.tile([128, C], bf16, name="wbf", tag="wbf")
            nc.vector.tensor_copy(wbf[:], wstg[:])
            for j in range(T):
                tp = pool_ab[i % 2].tile([128, 128], bf16, name="tp", tag="sA" if i % 2 == 0 else "sB")
                nc.tensor.transpose(tp[:], wbf[:, 128 * j:128 * (j + 1)], ident[:])
                nc.vector.tensor_copy(wTq[j][:, 128 * i:128 * (i + 1)], tp[:])
        for i in range(4):    # proj_w row-tiles
            wstg = wstg_pool.tile([128, C], f32, name="wstg", tag="wstg")
            nc.sync.dma_start(wstg[:], projw[128 * i:128 * (i + 1), :])
            wbf = wstg_pool.tile([128, C], bf16, name="wbf", tag="wbf")
            nc.vector.tensor_copy(wbf[:], wstg[:])
            for j in range(T):
                tp = pool_ab[i % 2].tile([128, 128], bf16, name="tp", tag="sA" if i % 2 == 0 else "sB")
                nc.tensor.transpose(tp[:], wbf[:, 128 * j:128 * (j + 1)], ident[:])
                nc.vector.tensor_copy(wTp[j][:, 128 * i:128 * (i + 1)], tp[:])

        # ---------------- input DMAs (x tiles 2-3 + consts) ----------------
        for t in range(2, T):
            for c4 in range(4):
                nc.sync.dma_start(
                    xf[t][:, 1024 * c4:1024 * (c4 + 1)],
                    xb[128 * t:128 * (t + 1), 1024 * c4:1024 * (c4 + 1)])
        for t in range(T):
            nc.sync.dma_start(nwt[t][:, 0:1], nw[128 * t:128 * (t + 1)])
            nc.sync.dma_start(nbt[t][:, 0:1], nb[128 * t:128 * (t + 1)])
            nc.sync.dma_start(pb[t][:, 0:1], projb[128 * t:128 * (t + 1)])
        for i in range(8):
            nc.sync.dma_start(qb[i][:, 0:1], qkvb[128 * i:128 * (i + 1)])
        # v bias broadcast to 128 partitions (stride-0 partition read)
        nc.gpsimd.dma_start(
            out=vb_bc[:],
            in_=bass.AP(tensor=qkvb.tensor, offset=1024, ap=[[0, 128], [1, C]]))
        nc.vector.memset(ones_row[0:1, :], 1.0)

        # ---------------- phase 1: group stats ----------------
        # chunked: reduce/square each 1024-col chunk as its DMA lands, so the
        # stats pipeline overlaps the x load instead of serializing after it
        # (tensor_reduce on a full [128,4096] tile is DVE's slowest op).
        spart = [left.tile([128, 8], f32, name=f"spart{t}", tag=f"spart{t}")
                 for t in range(T)]
        for t in range(T):
            for c4 in range(4):
                nc.vector.tensor_reduce(
                    out=spart[t][:, c4:c4 + 1],
                    in_=xf[t][:, 1024 * c4:1024 * (c4 + 1)],
                    axis=mybir.AxisListType.X, op=Alu.add)
                sq_scr = scr_pool.tile([128, 1024], bf16, name="sq_scr", tag="sq_scr")
                nc.scalar.activation(
                    sq_scr[:], xf[t][:, 1024 * c4:1024 * (c4 + 1)],
                    Act.Square, accum_out=spart[t][:, 4 + c4:5 + c4])
        for t in range(T):
            nc.vector.tensor_tensor(
                spart[t][:, 0:1], spart[t][:, 0:1], spart[t][:, 1:2], Alu.add)
            nc.vector.tensor_tensor(
                spart[t][:, 2:3], spart[t][:, 2:3], spart[t][:, 3:4], Alu.add)
            nc.vector.tensor_tensor(
                stat[t][:, 0:1], spart[t][:, 0:1], spart[t][:, 2:3], Alu.add)
            nc.vector.tensor_tensor(
                spart[t][:, 4:5], spart[t][:, 4:5], spart[t][:, 5:6], Alu.add)
            nc.vector.tensor_tensor(
                spart[t][:, 6:7], spart[t][:, 6:7], spart[t][:, 7:8], Alu.add)
            nc.vector.tensor_tensor(
                stat[t][:, 1:2], spart[t][:, 4:5], spart[t][:, 6:7], Alu.add)
            # group-reduce over partitions via indicator matmuls:
            # gg[g,s] = sum_ch ind[ch,g]*stat[ch,s]; then broadcast back
            gg_ps = psum_a.tile([2, 2], f32, name="gg_ps", tag="sA")
            nc.tensor.matmul(gg_ps[0:2, :], ind[:, 0:2], stat[t][:, 0:2],
                             start=True, stop=True)
            gg_sb = left.tile([2, 2], f32, name=f"gg_sb{t}", tag=f"gg_sb{t}")
            nc.vector.tensor_copy(gg_sb[0:2, :], gg_ps[0:2, :])
            gb_ps = psum_b.tile([128, 2], f32, name="gb_ps", tag="sB")
            nc.tensor.matmul(gb_ps[:, 0:2], indT[0:2, :], gg_sb[0:2, :],
                             start=True, stop=True)
            nc.vector.tensor_copy(gstat[t][:, 0:2], gb_ps[:, 0:2])
            # mean/var/rstd -> per-channel affine a,b
            mean_t = left.tile([128, 1], f32, name=f"mean{t}", tag=f"mean{t}")
            e2_t = left.tile([128, 1], f32, name=f"e2{t}", tag=f"e2{t}")
            var_t = left.tile([128, 1], f32, name=f"var{t}", tag=f"var{t}")
            std_t = left.tile([128, 1], f32, name=f"std{t}", tag=f"std{t}")
            a_t = left.tile([128, 1], f32, name=f"a{t}", tag=f"a{t}")
            b_t = left.tile([128, 1], f32, name=f"b{t}", tag=f"b{t}")
            inv = 1.0 / GELEM
            nc.vector.tensor_scalar(mean_t[:], gstat[t][:, 0:1], inv, None, Alu.mult)
            nc.vector.tensor_scalar(e2_t[:], gstat[t][:, 1:2], inv, None, Alu.mult)
            nc.vector.scalar_tensor_tensor(
                var_t[:], mean_t[:], -1.0, mean_t[:], Alu.mult, Alu.mult)
            nc.vector.scalar_tensor_tensor(
                var_t[:], e2_t[:], EPS, var_t[:], Alu.add, Alu.add)
            nc.scalar.activation(std_t[:], var_t[:], Act.Sqrt)
            nc.vector.reciprocal(a_t[:], std_t[:])
            nc.vector.tensor_tensor(a_t[:], a_t[:], nwt[t][:], Alu.mult)
            nc.vector.tensor_tensor(b_t[:], mean_t[:], a_t[:], Alu.mult)
            nc.vector.tensor_tensor(b_t[:], nbt[t][:], b_t[:], Alu.subtract)
            # phase 2: normalize + cast
            nc.vector.tensor_scalar(
                xn[t][:], xf[t][:], a_t[:, 0:1], b_t[:, 0:1], Alu.mult, Alu.add)

        right_ctx.close()

        # ---------------- mid pools (reuse xf space) ----------------
        mid = ctx.enter_context(tc.tile_pool(name="mid", bufs=1))
        psb_pool = ctx.enter_context(tc.tile_pool(name="psb_pool", bufs=4))
        rec_pool = ctx.enter_context(tc.tile_pool(name="rec_pool", bufs=2))
        yo_pool = ctx.enter_context(tc.tile_pool(name="yo_pool", bufs=2))

        vT = mid.tile([128, MT * 520], bf16, name="vT", tag="vT")
        yh = [mid.tile([128, 512], f32, name=f"yh{i}", tag=f"yh{i}") for i in range(T)]
        attn = [mid.tile([128, NQ], bf16, name=f"attn{t}", tag=f"attn{t}") for t in range(T)]
        xres = [mid.tile([128, NQ], f32, name=f"xres{t}", tag=f"xres{t}") for t in range(T)]
        for t in range(T):
            nc.sync.dma_start(xres[t][:], xb[128 * t:128 * (t + 1), 0:NQ])

        # ones columns of the augmented v^T (denominator trick)
        ones_view = vT[:].rearrange("p (m h x) -> p m h x", m=MT, x=65)[:, :, :, 64:65]
        nc.vector.memset(ones_view, 1.0)

        # ---------------- phase 3: projections ----------------
        # q: out rows 0..511 of qkv, only window-0 columns here; window-1 q
        # and k tiles 1..3 are emitted later as attention-period fillers.
        def q_chain(i, w, par):
            qp = pool_ab[par].tile([128, 512], f32, name="qp",
                                   tag="sA" if par == 0 else "sB")
            for k in range(T):
                nc.tensor.matmul(
                    qp[:], wTq[k][:, 128 * i:128 * i + 128],
                    xn[k][:, 512 * w:512 * w + 512],
                    start=(k == 0), stop=(k == T - 1))
            nc.vector.tensor_scalar(
                qsb[i][:, 512 * w:512 * w + 512], qp[:], qb[i][:, 0:1], None, Alu.add)

        def k_chain(i, w, par):
            kp = pool_ab[par].tile([128, 512], f32, name="kp",
                                   tag="sA" if par == 0 else "sB")
            for k in range(T):
                nc.tensor.matmul(
                    kp[:], wTq[k][:, 512 + 128 * i:512 + 128 * i + 128],
                    xn[k][:, 512 * w:512 * w + 512],
                    start=(k == 0), stop=(k == T - 1))
            nc.vector.tensor_scalar(
                ksb[i][:, 512 * w:512 * w + 512], kp[:], qb[4 + i][:, 0:1], None, Alu.add)

        def v_chain(mt, par):
            vp = pool_ab[par].tile([128, 512], f32, name="vp",
                                   tag="sA" if par == 0 else "sB")
            for k in range(T):
                nc.tensor.matmul(
                    vp[:], xn[k][:, 128 * mt:128 * mt + 128],
                    wTq[k][:, 1024:1536],
                    start=(k == 0), stop=(k == T - 1))
            dst = vT[:, 520 * mt:520 * mt + 520].rearrange(
                "p (h x) -> p h x", x=65)[:, :, 0:64]
            srcv = vp[:].rearrange("p (h x) -> p h x", x=64)
            vbv = vb_bc[:].rearrange("p (h x) -> p h x", x=64)
            nc.vector.tensor_tensor(dst, srcv, vbv, Alu.add)

        def proj_chain(i, w, par, ks=(0, 1, 2, 3), partial=None, combine=None):
            py = pool_ab[par].tile([128, 512], f32, name="py",
                                   tag="sA" if par == 0 else "sB")
            for n_, k in enumerate(ks):
                nc.tensor.matmul(
                    py[:], wTp[k][:, 128 * i:128 * i + 128],
                    attn[k][:, 512 * w:512 * w + 512],
                    start=(n_ == 0), stop=(n_ == len(ks) - 1))
            if partial is not None:
                nc.vector.tensor_copy(partial[:], py[:])
                return
            yo = yo_pool.tile([128, 512], f32, name="yo", tag="yo")
            nc.vector.scalar_tensor_tensor(
                yo[:], py[:], pb[i][:, 0:1], xres[i][:, 512 * w:512 * w + 512],
                Alu.add, Alu.add)
            if combine is not None:
                nc.vector.tensor_tensor(yo[:], yo[:], combine[:], Alu.add)
            nc.sync.dma_start(y[128 * i:128 * i + 128, 512 * w:512 * w + 512], yo[:])

        # prefix: k tiles 0-1, window-0 q, all of vT (PV consumes vT from
        # the first attention period on)
        for w8 in range(8):
            k_chain(0, w8, w8 % 2)
            k_chain(1, w8, (w8 + 1) % 2)
        for i in range(T):
            q_chain(i, 0, i % 2)
        for mt in range(3):
            v_chain(mt, mt % 2)

        # ---------------- phase 4: attention (flat pipelined stream) ------
        # Global stream of periods over (window, pair, group). PV runs one
        # period behind S/exp; pair normalize is deferred into the next
        # pair's first period; filler chains (k tiles 1-3, window-1 q,
        # window-0 proj) are emitted on alternate periods.
        def gsize(r):
            return 3 if r < NGRP - 1 else MT - 3 * (NGRP - 1)

        periods = [(w, p, r) for w in range(W) for p in range(4)
                   for r in range(NGRP)]
        pair_state = {}

        def emit_pv(w, p, r):
            pvs, ps_t = pair_state[(w, p)]
            if pvs[0] is None:
                for hh in range(2):
                    pvs[hh] = psum_pv.tile([128, 512], f32, name=f"pv{hh}", tag="pv")
            gs = gsize(r)
            for hh in range(2):
                h = 2 * p + hh
                pst = ps_t[hh][r]
                for j in range(gs):
                    m = 3 * r + j
                    nc.tensor.matmul(
                        pvs[hh][0:65, :],
                        vT[:, 520 * m + 65 * h:520 * m + 65 * h + 65],
                        pst[:, 512 * j:512 * j + 512],
                        start=(m == 0), stop=(m == MT - 1))

        def emit_normalize(w, p):
            pvs, _ = pair_state[(w, p)]
            for hh in range(2):
                h = 2 * p + hh
                kt, prr = h // 2, 64 * (h % 2)
                dnm = rec_pool.tile([1, 512], f32, name="dnm", tag="dnm")
                nc.vector.tensor_copy(dnm[0:1, :], pvs[hh][64:65, :])
                rec = rec_pool.tile([1, 512], f32, name="rec", tag="rec")
                rscr = rec_pool.tile([1, 512], f32, name="rscr", tag="rscr")
                nc.vector.reciprocal_approx_accurate(
                    rec[0:1, :], dnm[0:1, :], rscr[0:1, :])
                bc = pool_ab[hh].tile([64, 512], f32, name="bc",
                                     tag="sA" if hh == 0 else "sB")
                nc.tensor.matmul(
                    bc[0:64, :], ones_row[0:1, 0:D],
                    rec[0:1, :], start=True, stop=True)
                bcs = rec_pool.tile([64, 512], f32, name="bcs", tag="bcs")
                nc.vector.tensor_copy(bcs[0:64, :], bc[0:64, :])
                nc.vector.tensor_tensor(
                    attn[kt][prr:prr + 64, 512 * w:512 * w + 512],
                    pvs[hh][0:64, :], bcs[0:64, :], Alu.mult)

        # filler schedule: (earliest_period, closure); one pop on EVEN
        # periods, always from pool B (its next S alloc has ~2x more slack
        # than pool A's, so the filler's drain never delays the exp stream).
        # Emission deadlines: ksb[2] before period 22, ksb[3] before 33,
        # window-1 q before 44, window-0 proj after normalize(w0,p3) at 44.
        fillers = []
        for i in range(2, T):                       # ksb[2..3]: pops 2..16, 18..32
            for w8 in range(8):
                fillers.append((16 * (i - 2) + 2 + 2 * w8,
                                lambda i=i, w8=w8: k_chain(i, w8, 1)))
        for i in range(T):                          # window-1 q: pops 34..40
            fillers.append((34 + 2 * i, lambda i=i: q_chain(i, 1, 1)))
        for i in range(T):                          # window-0 proj: pops 46..52
            fillers.append((46 + 2 * i, lambda i=i: proj_chain(i, 0, 1)))
        for i in range(T):                          # w1 proj half (pairs 0-1)
            fillers.append((70 + 2 * i, lambda i=i:
                            proj_chain(i, 1, 1, ks=(0, 1), partial=yh[i])))
        fillers.reverse()   # pop from the end

        # ramp: vT m-tiles 3..31 emitted inside pair-0's periods, 3 per
        # period, always >= 2 m-tiles ahead of the PV stream that consumes
        # them (PV(r) reads vT[3r..3r+2] and is emitted at period r+1).
        ramp = [(max(0, (m - 5) // 3), (lambda m=m, par=m % 2: k_chain(0, 0, par)
                 if False else None)) for m in []]
        ramp = []
        for m in range(3, MT):
            ramp.append((max(0, (m - 5) // 3),
                         lambda m=m, par=m % 2: v_chain(m, par)))
        ramp.reverse()

        for g, (w, p, r) in enumerate(periods):
            gs = gsize(r)
            if r == 0:
                pair_state[(w, p)] = (
                    [None, None], [[None] * NGRP, [None] * NGRP])
            pvs, ps_t = pair_state[(w, p)]
            for hh in range(2):
                pr = 64 * hh
                sp = pool_ab[hh].tile([128, 512 * gs], f32, name=f"sp{hh}",
                                      tag="sA" if hh == 0 else "sB")
                for j in range(gs):
                    m = 3 * r + j
                    nc.tensor.matmul(
                        sp[:, 512 * j:512 * j + 512],
                        ksb[p][pr:pr + 64, 128 * m:128 * m + 128],
                        qsb[p][pr:pr + 64, 512 * w:512 * w + 512],
                        start=True, stop=True)
                pst = psb_pool.tile([128, 1536], bf16, name="ps", tag="ps")
                nc.scalar.activation(
                    pst[:, 0:512 * gs], sp[:, 0:512 * gs], Act.Exp, scale=0.125)
                ps_t[hh][r] = pst
                # PV of the previous period goes between the two S blocks
                # (h0 part) and after them (h1 part)
                if hh == 0:
                    if g > 0:
                        pw, pp, prr_ = periods[g - 1]
                        emit_pv(pw, pp, prr_)
            if g > 0 and periods[g - 1][2] == NGRP - 1:
                emit_normalize(*periods[g - 1][:2])
            popped = 0
            while ramp and ramp[-1][0] <= g and popped < 3:
                ramp.pop()[1]()
                popped += 1
            if not ramp and g % 2 == 0 and fillers and fillers[-1][0] <= g:
                fillers.pop()[1]()

        # tail: last period's PV, last normalize, window-1 proj second half
        emit_pv(*periods[-1])
        emit_normalize(W - 1, 3)
        while fillers:
            fillers.pop()[1]()
        for i in range(T):
            proj_chain(i, 1, i % 2, ks=(2, 3), combine=yh[i])


def _build():
    import concourse.tile as tile
    from concourse import bacc, mybir

    nc = bacc.Bacc("TRN2", target_bir_lowering=False, debug=False)
    f32 = mybir.dt.float32
    io = {
        "xb": nc.dram_tensor("xb", [C, N], f32, kind="ExternalInput").ap(),
        "qkvw": nc.dram_tensor("qkvw", [3 * C, C], f32, kind="ExternalInput").ap(),
        "qkvb": nc.dram_tensor("qkvb", [3 * C], f32, kind="ExternalInput").ap(),
        "projw": nc.dram_tensor("projw", [C, C], f32, kind="ExternalInput").ap(),
        "projb": nc.dram_tensor("projb", [C], f32, kind="ExternalInput").ap(),
        "nw": nc.dram_tensor("nw", [C], f32, kind="ExternalInput").ap(),
        "nb": nc.dram_tensor("nb", [C], f32, kind="ExternalInput").ap(),
        "cid": nc.dram_tensor("cid", [128, 128], mybir.dt.bfloat16,
                              kind="ExternalInput").ap(),
        "cind": nc.dram_tensor("cind", [128, 2], f32, kind="ExternalInput").ap(),
        "cindT": nc.dram_tensor("cindT", [2, 128], f32, kind="ExternalInput").ap(),
        "y": nc.dram_tensor("y", [C, NQ], f32, kind="ExternalOutput").ap(),
    }
    with tile.TileContext(nc) as tc:
        _emit(tc, io)
    nc.compile()
    return nc


def get_compiled():
    global _COMPILED
    if _COMPILED is None:
        _COMPILED = _build()
    return _COMPILED


def make_in_maps(x, norm_w, norm_b, qkv_w, qkv_b, proj_w, proj_b):
    import ml_dtypes

    xf = np.ascontiguousarray(np.asarray(x, np.float32)).reshape(2, C, N)
    ind = np.zeros((128, 2), np.float32)
    ind[0:64, 0] = 1.0
    ind[64:128, 1] = 1.0
    shared = {
        "cid": np.eye(128, dtype=ml_dtypes.bfloat16),
        "cind": ind,
        "cindT": np.ascontiguousarray(ind.T),
        "qkvw": np.ascontiguousarray(np.asarray(qkv_w, np.float32)),
        "qkvb": np.ascontiguousarray(np.asarray(qkv_b, np.float32)),
        "projw": np.ascontiguousarray(np.asarray(proj_w, np.float32)),
        "projb": np.ascontiguousarray(np.asarray(proj_b, np.float32)),
        "nw": np.ascontiguousarray(np.asarray(norm_w, np.float32)),
        "nb": np.ascontiguousarray(np.asarray(norm_b, np.float32)),
    }
    in_maps = []
    for core in range(8):
        bi, qs = core // 4, core % 4
        # rotate so this core's queries are always columns [0:NQ)
        xroll = np.concatenate(
            [xf[bi][:, qs * NQ:], xf[bi][:, :qs * NQ]], axis=1)
        m = dict(shared)
        m["xb"] = np.ascontiguousarray(xroll)
        in_maps.append(m)
    return in_maps


def assemble(results, x):
    y = np.zeros((2, C, N), np.float32)
    for core in range(8):
        bi, qs = core // 4, core % 4
        y[bi][:, qs * NQ:(qs + 1) * NQ] = results[core]["y"]
    return y.reshape(x.shape)


def kernel(x, norm_w, norm_b, qkv_w, qkv_b, proj_w, proj_b, **_ignored):
    from concourse import bass_utils

    nc = get_compiled()
    in_maps = make_in_maps(x, norm_w, norm_b, qkv_w, qkv_b, proj_w, proj_b)
    res = bass_utils.run_bass_kernel_spmd(nc, in_maps, core_ids=list(range(8)))
    return assemble(res.results, np.asarray(x))


# revision 17
# speedup vs baseline: 1.2068x; 1.2068x over previous
"""Trainium2 Bass kernel for nn_AttentionBlock (GroupNorm + MHA + proj + residual).

Full inputs in, full output out. Sharding: 8 cores = 2 batches x 4 query-slices.
Each core: GroupNorm over its batch image (replicated within the batch group),
q projection for its 1024 queries, k/v projections over all 4096 keys,
per-head attention (S^T = k^T q formulation, softmax along the PSUM partition
axis via an appended ones-column in the PV matmul), output projection and
residual for its query slice. Host side only slices/rotates/concatenates.

v2: phase 4 is software-pipelined per head-PAIR: the even head's S groups live
in a 3-bank PSUM pool A, the odd head's in pool B (plus 2 PV banks = 8).
Softmax exp runs as one N=1536 activation per group so ScalarE (the kernel's
throughput floor: ~2 exps of 16K elems per query-window per head) streams with
no gaps; PV matmuls are emitted one period behind S so the tensor engine FIFO
never stalls behind an exp dependency.

All matmuls run in bf16 with fp32 PSUM accumulation; softmax logits stay fp32.
"""
import numpy as np

C = 512          # channels
N = 4096         # pixels (64*64)
NQ = 1024        # queries per core
H = 8            # heads
D = 64           # head dim
T = 4            # 128-channel chunks
W = NQ // 512    # query windows of 512
MT = N // 128    # key m-tiles of 128
NGROUPS = 8
EPS = 1e-5
GELEM = (C // NGROUPS) * N   # elements per norm group
NGRP = 11                    # m-groups per head stream: [3]*10 + [2]

_COMPILED = None


def _emit(tc, io):
    import concourse.bass as bass
    from concourse import mybir
    from contextlib import ExitStack

    nc = tc.nc
    f32 = mybir.dt.float32
    bf16 = mybir.dt.bfloat16
    Alu = mybir.AluOpType
    Act = mybir.ActivationFunctionType

    xb, qkvw, qkvb, projw, projb, nw, nb, y = (
        io["xb"], io["qkvw"], io["qkvb"], io["projw"], io["projb"],
        io["nw"], io["nb"], io["y"])

    ctx = ExitStack()
    with ctx:
        # ---------------- pools ----------------
        # PSUM: pool A (3 banks) = even-head S stream, pool B (3 banks) =
        # odd-head S stream, pv pool 2x1 bank. 3+3+2 = 8 banks. Phase 1/3/5
        # transposes/projection chains borrow A/B between attention uses.
        left = ctx.enter_context(tc.tile_pool(name="left", bufs=1))
        psum_a = ctx.enter_context(tc.tile_pool(name="psum_a", bufs=1, space="PSUM"))
        psum_b = ctx.enter_context(tc.tile_pool(name="psum_b", bufs=1, space="PSUM"))
        psum_pv = ctx.enter_context(tc.tile_pool(name="psum_pv", bufs=2, space="PSUM"))
        pool_ab = [psum_a, psum_b]

        right_ctx = ExitStack()
        xf_pool = right_ctx.enter_context(
            tc.tile_pool(name="xf_pool", bufs=1, side="right"))
        wstg_pool = right_ctx.enter_context(
            tc.tile_pool(name="wstg_pool", bufs=4, side="right"))
        scr_pool = right_ctx.enter_context(
            tc.tile_pool(name="scr_pool", bufs=2, side="right"))

        # ---------------- persistent tiles ----------------
        xn = [left.tile([128, N], bf16, name=f"xn{t}", tag=f"xn{t}") for t in range(T)]
        ksb = [left.tile([128, N], bf16, name=f"ksb{t}", tag=f"ksb{t}") for t in range(T)]
        qsb = [left.tile([128, NQ], bf16, name=f"qsb{t}", tag=f"qsb{t}") for t in range(T)]
        wTq = [left.tile([128, 1536], bf16, name=f"wTq{t}", tag=f"wTq{t}") for t in range(T)]
        wTp = [left.tile([128, C], bf16, name=f"wTp{t}", tag=f"wTp{t}") for t in range(T)]
        vb_bc = left.tile([128, C], f32, name="vb_bc", tag="vb_bc")
        ones_row = left.tile([1, D], f32, name="ones_row", tag="ones_row")
        qb = [left.tile([128, 1], f32, name=f"qb{i}", tag=f"qb{i}") for i in range(8)]
        pb = [left.tile([128, 1], f32, name=f"pb{i}", tag=f"pb{i}") for i in range(T)]
        nwt = [left.tile([128, 1], f32, name=f"nwt{t}", tag=f"nwt{t}") for t in range(T)]
        nbt = [left.tile([128, 1], f32, name=f"nbt{t}", tag=f"nbt{t}") for t in range(T)]
        stat = [left.tile([128, 2], f32, name=f"stat{t}", tag=f"stat{t}") for t in range(T)]
        gstat = [left.tile([128, 2], f32, name=f"gstat{t}", tag=f"gstat{t}") for t in range(T)]

        # ---------------- input DMAs (x tiles 0-1 first) ----------------
        xf = [xf_pool.tile([128, N], f32, name=f"xf{t}", tag=f"xf{t}") for t in range(T)]
        for t in range(2):
            for c4 in range(4):
                nc.sync.dma_start(
                    xf[t][:, 1024 * c4:1024 * (c4 + 1)],
                    xb[128 * t:128 * (t + 1), 1024 * c4:1024 * (c4 + 1)])
        # weights: natural-layout contiguous DMA, cast to bf16, transpose
        # 128x128 blocks on the PE (identity trick) into wTq/wTp.
        ident = left.tile([128, 128], bf16, name="ident", tag="ident")
        nc.sync.dma_start(ident[:], io["cid"][:, :])
        ind = left.tile([128, 2], f32, name="ind", tag="ind")
        nc.sync.dma_start(ind[:], io["cind"][:, :])
        indT = left.tile([2, 128], f32, name="indT", tag="indT")
        nc.sync.dma_start(indT[0:2, :], io["cindT"][:, :])
        for i in range(12):   # qkv_w row-tiles
            wstg = wstg_pool.tile([128, C], f32, name="wstg", tag="wstg")
            nc.sync.dma_start(wstg[:], qkvw[128 * i:128 * (i + 1), :])
            wbf = wstg_pool.tile([128, C], bf16, name="wbf", tag="wbf")
            nc.vector.tensor_copy(wbf[:], wstg[:])
            for j in range(T):
                tp = pool_ab[i % 2].tile([128, 128], bf16, name="tp", tag="sA" if i % 2 == 0 else "sB")
                nc.tensor.transpose(tp[:], wbf[:, 128 * j:128 * (j + 1)], ident[:])
                nc.vector.tensor_copy(wTq[j][:, 128 * i:128 * (i + 1)], tp[:])
        for i in range(4):    # proj_w row-tiles
            wstg = wstg_pool.tile([128, C], f32, name="wstg", tag="wstg")
            nc.sync.dma_start(wstg[:], projw[128 * i:128 * (i + 1), :])
            wbf = wstg_pool.tile([128, C], bf16, name="wbf", tag="wbf")
            nc.vector.tensor_copy(wbf[:], wstg[:])
            for j in range(T):
                tp = pool_ab[i % 2].tile([128, 128], bf16, name="tp", tag="sA" if i % 2 == 0 else "sB")
                nc.tensor.transpose(tp[:], wbf[:, 128 * j:128 * (j + 1)], ident[:])
                nc.vector.tensor_copy(wTp[j][:, 128 * i:128 * (i + 1)], tp[:])

        # ---------------- input DMAs (x tiles 2-3 + consts) ----------------
        for t in range(2, T):
            for c4 in range(4):
                nc.sync.dma_start(
                    xf[t][:, 1024 * c4:1024 * (c4 + 1)],
                    xb[128 * t:128 * (t + 1), 1024 * c4:1024 * (c4 + 1)])
        for t in range(T):
            nc.sync.dma_start(nwt[t][:, 0:1], nw[128 * t:128 * (t + 1)])
            nc.sync.dma_start(nbt[t][:, 0:1], nb[128 * t:128 * (t + 1)])
            nc.sync.dma_start(pb[t][:, 0:1], projb[128 * t:128 * (t + 1)])
        for i in range(8):
            nc.sync.dma_start(qb[i][:, 0:1], qkvb[128 * i:128 * (i + 1)])
        # v bias broadcast to 128 partitions (stride-0 partition read)
        nc.gpsimd.dma_start(
            out=vb_bc[:],
            in_=bass.AP(tensor=qkvb.tensor, offset=1024, ap=[[0, 128], [1, C]]))
        nc.vector.memset(ones_row[0:1, :], 1.0)

        # ---------------- phase 1: group stats ----------------
        # chunked: reduce/square each 1024-col chunk as its DMA lands, so the
        # stats pipeline overlaps the x load instead of serializing after it
        # (tensor_reduce on a full [128,4096] tile is DVE's slowest op).
        spart = [left.tile([128, 8], f32, name=f"spart{t}", tag=f"spart{t}")
                 for t in range(T)]
        for t in range(T):
            for c4 in range(4):
                nc.vector.tensor_reduce(
                    out=spart[t][:, c4:c4 + 1],
                    in_=xf[t][:, 1024 * c4:1024 * (c4 + 1)],
                    axis=mybir.AxisListType.X, op=Alu.add)
                sq_scr = scr_pool.tile([128, 1024], bf16, name="sq_scr", tag="sq_scr")
                nc.scalar.activation(
                    sq_scr[:], xf[t][:, 1024 * c4:1024 * (c4 + 1)],
                    Act.Square, accum_out=spart[t][:, 4 + c4:5 + c4])
        for t in range(T):
            nc.vector.tensor_tensor(
                spart[t][:, 0:1], spart[t][:, 0:1], spart[t][:, 1:2], Alu.add)
            nc.vector.tensor_tensor(
                spart[t][:, 2:3], spart[t][:, 2:3], spart[t][:, 3:4], Alu.add)
            nc.vector.tensor_tensor(
                stat[t][:, 0:1], spart[t][:, 0:1], spart[t][:, 2:3], Alu.add)
            nc.vector.tensor_tensor(
                spart[t][:, 4:5], spart[t][:, 4:5], spart[t][:, 5:6], Alu.add)
            nc.vector.tensor_tensor(
                spart[t][:, 6:7], spart[t][:, 6:7], spart[t][:, 7:8], Alu.add)
            nc.vector.tensor_tensor(
                stat[t][:, 1:2], spart[t][:, 4:5], spart[t][:, 6:7], Alu.add)
            # group-reduce over partitions via indicator matmuls:
            # gg[g,s] = sum_ch ind[ch,g]*stat[ch,s]; then broadcast back
            gg_ps = psum_a.tile([2, 2], f32, name="gg_ps", tag="sA")
            nc.tensor.matmul(gg_ps[0:2, :], ind[:, 0:2], stat[t][:, 0:2],
                             start=True, stop=True)
            gg_sb = left.tile([2, 2], f32, name=f"gg_sb{t}", tag=f"gg_sb{t}")
            nc.vector.tensor_copy(gg_sb[0:2, :], gg_ps[0:2, :])
            gb_ps = psum_b.tile([128, 2], f32, name="gb_ps", tag="sB")
            nc.tensor.matmul(gb_ps[:, 0:2], indT[0:2, :], gg_sb[0:2, :],
                             start=True, stop=True)
            nc.vector.tensor_copy(gstat[t][:, 0:2], gb_ps[:, 0:2])
            # mean/var/rstd -> per-channel affine a,b
            mean_t = left.tile([128, 1], f32, name=f"mean{t}", tag=f"mean{t}")
            e2_t = left.tile([128, 1], f32, name=f"e2{t}", tag=f"e2{t}")
            var_t = left.tile([128, 1], f32, name=f"var{t}", tag=f"var{t}")
            std_t = left.tile([128, 1], f32, name=f"std{t}", tag=f"std{t}")
            a_t = left.tile([128, 1], f32, name=f"a{t}", tag=f"a{t}")
            b_t = left.tile([128, 1], f32, name=f"b{t}", tag=f"b{t}")
            inv = 1.0 / GELEM
            nc.vector.tensor_scalar(mean_t[:], gstat[t][:, 0:1], inv, None, Alu.mult)
            nc.vector.tensor_scalar(e2_t[:], gstat[t][:, 1:2], inv, None, Alu.mult)
            nc.vector.scalar_tensor_tensor(
                var_t[:], mean_t[:], -1.0, mean_t[:], Alu.mult, Alu.mult)
            nc.vector.scalar_tensor_tensor(
                var_t[:], e2_t[:], EPS, var_t[:], Alu.add, Alu.add)
            nc.scalar.activation(std_t[:], var_t[:], Act.Sqrt)
            nc.vector.reciprocal(a_t[:], std_t[:])
            nc.vector.tensor_tensor(a_t[:], a_t[:], nwt[t][:], Alu.mult)
            nc.vector.tensor_tensor(b_t[:], mean_t[:], a_t[:], Alu.mult)
            nc.vector.tensor_tensor(b_t[:], nbt[t][:], b_t[:], Alu.subtract)
            # phase 2: normalize + cast
            nc.vector.tensor_scalar(
                xn[t][:], xf[t][:], a_t[:, 0:1], b_t[:, 0:1], Alu.mult, Alu.add)

        right_ctx.close()

        # ---------------- mid pools (reuse xf space) ----------------
        mid = ctx.enter_context(tc.tile_pool(name="mid", bufs=1))
        psb_pool = ctx.enter_context(tc.tile_pool(name="psb_pool", bufs=4))
        rec_pool = ctx.enter_context(tc.tile_pool(name="rec_pool", bufs=2))
        yo_pool = ctx.enter_context(tc.tile_pool(name="yo_pool", bufs=2))

        vT = mid.tile([128, MT * 520], bf16, name="vT", tag="vT")
        yh = [mid.tile([128, 512], f32, name=f"yh{i}", tag=f"yh{i}") for i in range(T)]
        attn = [mid.tile([128, NQ], bf16, name=f"attn{t}", tag=f"attn{t}") for t in range(T)]
        xres = [mid.tile([128, NQ], f32, name=f"xres{t}", tag=f"xres{t}") for t in range(T)]
        for t in range(T):
            nc.sync.dma_start(xres[t][:], xb[128 * t:128 * (t + 1), 0:NQ])

        # ones columns of the augmented v^T (denominator trick)
        ones_view = vT[:].rearrange("p (m h x) -> p m h x", m=MT, x=65)[:, :, :, 64:65]
        nc.vector.memset(ones_view, 1.0)

        # ---------------- phase 3: projections ----------------
        # q: out rows 0..511 of qkv, only window-0 columns here; window-1 q
        # and k tiles 1..3 are emitted later as attention-period fillers.
        def q_chain(i, w, par):
            qp = pool_ab[par].tile([128, 512], f32, name="qp",
                                   tag="sA" if par == 0 else "sB")
            for k in range(T):
                nc.tensor.matmul(
                    qp[:], wTq[k][:, 128 * i:128 * i + 128],
                    xn[k][:, 512 * w:512 * w + 512],
                    start=(k == 0), stop=(k == T - 1))
            nc.vector.tensor_scalar(
                qsb[i][:, 512 * w:512 * w + 512], qp[:], qb[i][:, 0:1], None, Alu.add)

        def k_chain(i, w, par):
            kp = pool_ab[par].tile([128, 512], f32, name="kp",
                                   tag="sA" if par == 0 else "sB")
            for k in range(T):
                nc.tensor.matmul(
                    kp[:], wTq[k][:, 512 + 128 * i:512 + 128 * i + 128],
                    xn[k][:, 512 * w:512 * w + 512],
                    start=(k == 0), stop=(k == T - 1))
            nc.vector.tensor_scalar(
                ksb[i][:, 512 * w:512 * w + 512], kp[:], qb[4 + i][:, 0:1], None, Alu.add)

        def v_chain(mt, par):
            vp = pool_ab[par].tile([128, 512], f32, name="vp",
                                   tag="sA" if par == 0 else "sB")
            for k in range(T):
                nc.tensor.matmul(
                    vp[:], xn[k][:, 128 * mt:128 * mt + 128],
                    wTq[k][:, 1024:1536],
                    start=(k == 0), stop=(k == T - 1))
            dst = vT[:, 520 * mt:520 * mt + 520].rearrange(
                "p (h x) -> p h x", x=65)[:, :, 0:64]
            srcv = vp[:].rearrange("p (h x) -> p h x", x=64)
            vbv = vb_bc[:].rearrange("p (h x) -> p h x", x=64)
            nc.vector.tensor_tensor(dst, srcv, vbv, Alu.add)

        def proj_chain(i, w, par, ks=(0, 1, 2, 3), partial=None, combine=None):
            py = pool_ab[par].tile([128, 512], f32, name="py",
                                   tag="sA" if par == 0 else "sB")
            for n_, k in enumerate(ks):
                nc.tensor.matmul(
                    py[:], wTp[k][:, 128 * i:128 * i + 128],
                    attn[k][:, 512 * w:512 * w + 512],
                    start=(n_ == 0), stop=(n_ == len(ks) - 1))
            if partial is not None:
                nc.vector.tensor_copy(partial[:], py[:])
                return
            yo = yo_pool.tile([128, 512], f32, name="yo", tag="yo")
            nc.vector.scalar_tensor_tensor(
                yo[:], py[:], pb[i][:, 0:1], xres[i][:, 512 * w:512 * w + 512],
                Alu.add, Alu.add)
            if combine is not None:
                nc.vector.tensor_tensor(yo[:], yo[:], combine[:], Alu.add)
            nc.sync.dma_start(y[128 * i:128 * i + 128, 512 * w:512 * w + 512], yo[:])

        # prefix: k tiles 0-1, window-0 q, all of vT (PV consumes vT from
        # the first attention period on)
        for w8 in range(8):
            k_chain(0, w8, w8 % 2)
            k_chain(1, w8, (w8 + 1) % 2)
        for i in range(T):
            q_chain(i, 0, i % 2)
        for mt in range(MT):
            v_chain(mt, mt % 2)

        # ---------------- phase 4: attention (flat pipelined stream) ------
        # Global stream of periods over (window, pair, group). PV runs one
        # period behind S/exp; pair normalize is deferred into the next
        # pair's first period; filler chains (k tiles 1-3, window-1 q,
        # window-0 proj) are emitted on alternate periods.
        def gsize(r):
            return 3 if r < NGRP - 1 else MT - 3 * (NGRP - 1)

        periods = [(w, p, r) for w in range(W) for p in range(4)
                   for r in range(NGRP)]
        pair_state = {}

        def emit_pv(w, p, r):
            pvs, ps_t = pair_state[(w, p)]
            if pvs[0] is None:
                for hh in range(2):
                    pvs[hh] = psum_pv.tile([128, 512], f32, name=f"pv{hh}", tag="pv")
            gs = gsize(r)
            for hh in range(2):
                h = 2 * p + hh
                pst = ps_t[hh][r]
                for j in range(gs):
                    m = 3 * r + j
                    nc.tensor.matmul(
                        pvs[hh][0:65, :],
                        vT[:, 520 * m + 65 * h:520 * m + 65 * h + 65],
                        pst[:, 512 * j:512 * j + 512],
                        start=(m == 0), stop=(m == MT - 1))

        def emit_normalize(w, p):
            pvs, _ = pair_state[(w, p)]
            for hh in range(2):
                h = 2 * p + hh
                kt, prr = h // 2, 64 * (h % 2)
                pvc = rec_pool.tile([65, 512], f32, name="pvc", tag="pvc")
                nc.vector.tensor_copy(pvc[0:65, :], pvs[hh][0:65, :])
                dnm = rec_pool.tile([1, 512], f32, name="dnm", tag="dnm")
                nc.vector.tensor_copy(dnm[0:1, :], pvc[64:65, :])
                rec = rec_pool.tile([1, 512], f32, name="rec", tag="rec")
                rscr = rec_pool.tile([1, 512], f32, name="rscr", tag="rscr")
                nc.vector.reciprocal_approx_accurate(
                    rec[0:1, :], dnm[0:1, :], rscr[0:1, :])
                bc = pool_ab[hh].tile([64, 512], f32, name="bc",
                                     tag="sA" if hh == 0 else "sB")
                nc.tensor.matmul(
                    bc[0:64, :], ones_row[0:1, 0:D],
                    rec[0:1, :], start=True, stop=True)
                bcs = rec_pool.tile([64, 512], f32, name="bcs", tag="bcs")
                nc.vector.tensor_copy(bcs[0:64, :], bc[0:64, :])
                nc.vector.tensor_tensor(
                    attn[kt][prr:prr + 64, 512 * w:512 * w + 512],
                    pvc[0:64, :], bcs[0:64, :], Alu.mult)

        # filler schedule: (earliest_period, closure); one pop on EVEN
        # periods, always from pool B (its next S alloc has ~2x more slack
        # than pool A's, so the filler's drain never delays the exp stream).
        # Emission deadlines: ksb[2] before period 22, ksb[3] before 33,
        # window-1 q before 44, window-0 proj after normalize(w0,p3) at 44.
        fillers = []
        for i in range(2, T):                       # ksb[2..3]: pops 2..16, 18..32
            for w8 in range(8):
                fillers.append((16 * (i - 2) + 2 + 2 * w8,
                                lambda i=i, w8=w8: k_chain(i, w8, 1)))
        for i in range(T):                          # window-1 q: pops 34..40
            fillers.append((34 + 2 * i, lambda i=i: q_chain(i, 1, 1)))
        for i in range(T):                          # window-0 proj: pops 46..52
            fillers.append((46 + 2 * i, lambda i=i: proj_chain(i, 0, 1)))
        for i in range(T):                          # w1 proj half (pairs 0-1)
            fillers.append((70 + 2 * i, lambda i=i:
                            proj_chain(i, 1, 1, ks=(0, 1), partial=yh[i])))
        fillers.reverse()   # pop from the end

        for g, (w, p, r) in enumerate(periods):
            gs = gsize(r)
            if r == 0:
                pair_state[(w, p)] = (
                    [None, None], [[None] * NGRP, [None] * NGRP])
            pvs, ps_t = pair_state[(w, p)]
            for hh in range(2):
                pr = 64 * hh
                sp = pool_ab[hh].tile([128, 512 * gs], f32, name=f"sp{hh}",
                                      tag="sA" if hh == 0 else "sB")
                for j in range(gs):
                    m = 3 * r + j
                    nc.tensor.matmul(
                        sp[:, 512 * j:512 * j + 512],
                        ksb[p][pr:pr + 64, 128 * m:128 * m + 128],
                        qsb[p][pr:pr + 64, 512 * w:512 * w + 512],
                        start=True, stop=True)
                pst = psb_pool.tile([128, 1536], bf16, name="ps", tag="ps")
                nc.scalar.activation(
                    pst[:, 0:512 * gs], sp[:, 0:512 * gs], Act.Exp, scale=0.125)
                ps_t[hh][r] = pst
                # PV of the previous period goes between the two S blocks
                # (h0 part) and after them (h1 part)
                if hh == 0:
                    if g > 0:
                        pw, pp, prr_ = periods[g - 1]
                        emit_pv(pw, pp, prr_)
            if g > 0 and periods[g - 1][2] == NGRP - 1:
                emit_normalize(*periods[g - 1][:2])
            if g % 2 == 0 and fillers and fillers[-1][0] <= g:
                fillers.pop()[1]()

        # tail: last period's PV, last normalize, window-1 proj second half
        emit_pv(*periods[-1])
        emit_normalize(W - 1, 3)
        while fillers:
            fillers.pop()[1]()
        for i in range(T):
            proj_chain(i, 1, i % 2, ks=(2, 3), combine=yh[i])


def _build():
    import concourse.tile as tile
    from concourse import bacc, mybir

    nc = bacc.Bacc("TRN2", target_bir_lowering=False, debug=False)
    f32 = mybir.dt.float32
    io = {
        "xb": nc.dram_tensor("xb", [C, N], f32, kind="ExternalInput").ap(),
        "qkvw": nc.dram_tensor("qkvw", [3 * C, C], f32, kind="ExternalInput").ap(),
        "qkvb": nc.dram_tensor("qkvb", [3 * C], f32, kind="ExternalInput").ap(),
        "projw": nc.dram_tensor("projw", [C, C], f32, kind="ExternalInput").ap(),
        "projb": nc.dram_tensor("projb", [C], f32, kind="ExternalInput").ap(),
        "nw": nc.dram_tensor("nw", [C], f32, kind="ExternalInput").ap(),
        "nb": nc.dram_tensor("nb", [C], f32, kind="ExternalInput").ap(),
        "cid": nc.dram_tensor("cid", [128, 128], mybir.dt.bfloat16,
                              kind="ExternalInput").ap(),
        "cind": nc.dram_tensor("cind", [128, 2], f32, kind="ExternalInput").ap(),
        "cindT": nc.dram_tensor("cindT", [2, 128], f32, kind="ExternalInput").ap(),
        "y": nc.dram_tensor("y", [C, NQ], f32, kind="ExternalOutput").ap(),
    }
    with tile.TileContext(nc) as tc:
        _emit(tc, io)
    nc.compile()
    return nc


def get_compiled():
    global _COMPILED
    if _COMPILED is None:
        _COMPILED = _build()
    return _COMPILED


def make_in_maps(x, norm_w, norm_b, qkv_w, qkv_b, proj_w, proj_b):
    import ml_dtypes

    xf = np.ascontiguousarray(np.asarray(x, np.float32)).reshape(2, C, N)
    ind = np.zeros((128, 2), np.float32)
    ind[0:64, 0] = 1.0
    ind[64:128, 1] = 1.0
    shared = {
        "cid": np.eye(128, dtype=ml_dtypes.bfloat16),
        "cind": ind,
        "cindT": np.ascontiguousarray(ind.T),
        "qkvw": np.ascontiguousarray(np.asarray(qkv_w, np.float32)),
        "qkvb": np.ascontiguousarray(np.asarray(qkv_b, np.float32)),
        "projw": np.ascontiguousarray(np.asarray(proj_w, np.float32)),
        "projb": np.ascontiguousarray(np.asarray(proj_b, np.float32)),
        "nw": np.ascontiguousarray(np.asarray(norm_w, np.float32)),
        "nb": np.ascontiguousarray(np.asarray(norm_b, np.float32)),
    }
    in_maps = []
    for core in range(8):
        bi, qs = core // 4, core % 4
        # rotate so this core's queries are always columns [0:NQ)
        xroll = np.concatenate(
            [xf[bi][:, qs * NQ:], xf[bi][:, :qs * NQ]], axis=1)
        m = dict(shared)
        m["xb"] = np.ascontiguousarray(xroll)
        in_maps.append(m)
    return in_maps


def assemble(results, x):
    y = np.zeros((2, C, N), np.float32)
    for core in range(8):
        bi, qs = core // 4, core % 4
        y[bi][:, qs * NQ:(qs + 1) * NQ] = results[core]["y"]
    return y.reshape(x.shape)


def kernel(x, norm_w, norm_b, qkv_w, qkv_b, proj_w, proj_b, **_ignored):
    from concourse import bass_utils

    nc = get_compiled()
    in_maps = make_in_maps(x, norm_w, norm_b, qkv_w, qkv_b, proj_w, proj_b)
    res = bass_utils.run_bass_kernel_spmd(nc, in_maps, core_ids=list(range(8)))
    return assemble(res.results, np.asarray(x))


# revision 21
# speedup vs baseline: 1.2732x; 1.0550x over previous
"""Trainium2 Bass kernel for nn_AttentionBlock (GroupNorm + MHA + proj + residual).

Full inputs in, full output out. Sharding: 8 cores = 2 batches x 4 query-slices.
Each core: GroupNorm over its batch image (replicated within the batch group),
q projection for its 1024 queries, k/v projections over all 4096 keys,
per-head attention (S^T = k^T q formulation, softmax along the PSUM partition
axis via an appended ones-column in the PV matmul), output projection and
residual for its query slice. Host side only slices/rotates/concatenates.

v2: phase 4 is software-pipelined per head-PAIR: the even head's S groups live
in a 3-bank PSUM pool A, the odd head's in pool B (plus 2 PV banks = 8).
Softmax exp runs as one N=1536 activation per group so ScalarE (the kernel's
throughput floor: ~2 exps of 16K elems per query-window per head) streams with
no gaps; PV matmuls are emitted one period behind S so the tensor engine FIFO
never stalls behind an exp dependency.

All matmuls run in bf16 with fp32 PSUM accumulation; softmax logits stay fp32.
"""
import numpy as np

C = 512          # channels
N = 4096         # pixels (64*64)
NQ = 1024        # queries per core
H = 8            # heads
D = 64           # head dim
T = 4            # 128-channel chunks
W = NQ // 512    # query windows of 512
MT = N // 128    # key m-tiles of 128
NGROUPS = 8
EPS = 1e-5
GELEM = (C // NGROUPS) * N   # elements per norm group
NGRP = 11                    # m-groups per head stream: [3]*10 + [2]

_COMPILED = None


def _emit(tc, io):
    import concourse.bass as bass
    from concourse import mybir
    from contextlib import ExitStack

    nc = tc.nc
    f32 = mybir.dt.float32
    bf16 = mybir.dt.bfloat16
    Alu = mybir.AluOpType
    Act = mybir.ActivationFunctionType

    xb, qkvw, qkvb, projw, projb, nw, nb, y = (
        io["xb"], io["qkvw"], io["qkvb"], io["projw"], io["projb"],
        io["nw"], io["nb"], io["y"])

    ctx = ExitStack()
    with ctx:
        # ---------------- pools ----------------
        # PSUM: pool A (3 banks) = even-head S stream, pool B (3 banks) =
        # odd-head S stream, pv pool 2x1 bank. 3+3+2 = 8 banks. Phase 1/3/5
        # transposes/projection chains borrow A/B between attention uses.
        left = ctx.enter_context(tc.tile_pool(name="left", bufs=1))
        psum_a = ctx.enter_context(tc.tile_pool(name="psum_a", bufs=1, space="PSUM"))
        psum_b = ctx.enter_context(tc.tile_pool(name="psum_b", bufs=1, space="PSUM"))
        psum_pv = ctx.enter_context(tc.tile_pool(name="psum_pv", bufs=2, space="PSUM"))
        pool_ab = [psum_a, psum_b]

        right_ctx = ExitStack()
        xf_pool = right_ctx.enter_context(
            tc.tile_pool(name="xf_pool", bufs=1, side="right"))
        wstg_pool = right_ctx.enter_context(
            tc.tile_pool(name="wstg_pool", bufs=4, side="right"))
        scr_pool = right_ctx.enter_context(
            tc.tile_pool(name="scr_pool", bufs=2, side="right"))

        # ---------------- persistent tiles ----------------
        xn = [left.tile([128, N], bf16, name=f"xn{t}", tag=f"xn{t}") for t in range(T)]
        ksb = [left.tile([128, N], bf16, name=f"ksb{t}", tag=f"ksb{t}") for t in range(T)]
        qsb = [left.tile([128, NQ], bf16, name=f"qsb{t}", tag=f"qsb{t}") for t in range(T)]
        wTq = [left.tile([128, 1536], bf16, name=f"wTq{t}", tag=f"wTq{t}") for t in range(T)]
        wTp = [left.tile([128, C], bf16, name=f"wTp{t}", tag=f"wTp{t}") for t in range(T)]
        vb_bc = left.tile([128, C], f32, name="vb_bc", tag="vb_bc")
        ones_row = left.tile([1, D], f32, name="ones_row", tag="ones_row")
        qb = [left.tile([128, 1], f32, name=f"qb{i}", tag=f"qb{i}") for i in range(8)]
        pb = [left.tile([128, 1], f32, name=f"pb{i}", tag=f"pb{i}") for i in range(T)]
        nwt = [left.tile([128, 1], f32, name=f"nwt{t}", tag=f"nwt{t}") for t in range(T)]
        nbt = [left.tile([128, 1], f32, name=f"nbt{t}", tag=f"nbt{t}") for t in range(T)]
        stat = [left.tile([128, 2], f32, name=f"stat{t}", tag=f"stat{t}") for t in range(T)]
        gstat = [left.tile([128, 2], f32, name=f"gstat{t}", tag=f"gstat{t}") for t in range(T)]

        # ---------------- input DMAs (x tiles 0-1 first) ----------------
        xf = [xf_pool.tile([128, N], f32, name=f"xf{t}", tag=f"xf{t}") for t in range(T)]
        for t in range(2):
            for c4 in range(4):
                eng = nc.sync if c4 % 2 == 0 else nc.scalar
                eng.dma_start(
                    xf[t][:, 1024 * c4:1024 * (c4 + 1)],
                    xb[128 * t:128 * (t + 1), 1024 * c4:1024 * (c4 + 1)])
        # weights: natural-layout contiguous DMA, cast to bf16, transpose
        # 128x128 blocks on the PE (identity trick) into wTq/wTp.
        ident = left.tile([128, 128], bf16, name="ident", tag="ident")
        nc.sync.dma_start(ident[:], io["cid"][:, :])
        ind = left.tile([128, 2], f32, name="ind", tag="ind")
        nc.sync.dma_start(ind[:], io["cind"][:, :])
        indT = left.tile([2, 128], f32, name="indT", tag="indT")
        nc.sync.dma_start(indT[0:2, :], io["cindT"][:, :])
        for i in range(12):   # qkv_w row-tiles
            wstg = wstg_pool.tile([128, C], f32, name="wstg", tag="wstg")
            nc.sync.dma_start(wstg[:], qkvw[128 * i:128 * (i + 1), :])
            wbf = wstg_pool.tile([128, C], bf16, name="wbf", tag="wbf")
            nc.vector.tensor_copy(wbf[:], wstg[:])
            for j in range(T):
                tp = pool_ab[i % 2].tile([128, 128], bf16, name="tp", tag="sA" if i % 2 == 0 else "sB")
                nc.tensor.transpose(tp[:], wbf[:, 128 * j:128 * (j + 1)], ident[:])
                nc.vector.tensor_copy(wTq[j][:, 128 * i:128 * (i + 1)], tp[:])
        for i in range(4):    # proj_w row-tiles
            wstg = wstg_pool.tile([128, C], f32, name="wstg", tag="wstg")
            nc.sync.dma_start(wstg[:], projw[128 * i:128 * (i + 1), :])
            wbf = wstg_pool.tile([128, C], bf16, name="wbf", tag="wbf")
            nc.vector.tensor_copy(wbf[:], wstg[:])
            for j in range(T):
                tp = pool_ab[i % 2].tile([128, 128], bf16, name="tp", tag="sA" if i % 2 == 0 else "sB")
                nc.tensor.transpose(tp[:], wbf[:, 128 * j:128 * (j + 1)], ident[:])
                nc.vector.tensor_copy(wTp[j][:, 128 * i:128 * (i + 1)], tp[:])

        # ---------------- input DMAs (x tiles 2-3 + consts) ----------------
        for t in range(2, T):
            for c4 in range(4):
                eng = nc.sync if c4 % 2 == 0 else nc.scalar
                eng.dma_start(
                    xf[t][:, 1024 * c4:1024 * (c4 + 1)],
                    xb[128 * t:128 * (t + 1), 1024 * c4:1024 * (c4 + 1)])
        for t in range(T):
            nc.sync.dma_start(nwt[t][:, 0:1], nw[128 * t:128 * (t + 1)])
            nc.sync.dma_start(nbt[t][:, 0:1], nb[128 * t:128 * (t + 1)])
            nc.sync.dma_start(pb[t][:, 0:1], projb[128 * t:128 * (t + 1)])
        for i in range(8):
            nc.sync.dma_start(qb[i][:, 0:1], qkvb[128 * i:128 * (i + 1)])
        # v bias broadcast to 128 partitions (stride-0 partition read)
        nc.gpsimd.dma_start(
            out=vb_bc[:],
            in_=bass.AP(tensor=qkvb.tensor, offset=1024, ap=[[0, 128], [1, C]]))
        nc.vector.memset(ones_row[0:1, :], 1.0)

        # ---------------- phase 1: group stats ----------------
        # chunked: reduce/square each 1024-col chunk as its DMA lands, so the
        # stats pipeline overlaps the x load instead of serializing after it
        # (tensor_reduce on a full [128,4096] tile is DVE's slowest op).
        spart = [left.tile([128, 8], f32, name=f"spart{t}", tag=f"spart{t}")
                 for t in range(T)]
        for t in range(T):
            for c4 in range(4):
                nc.vector.tensor_reduce(
                    out=spart[t][:, c4:c4 + 1],
                    in_=xf[t][:, 1024 * c4:1024 * (c4 + 1)],
                    axis=mybir.AxisListType.X, op=Alu.add)
                sq_scr = scr_pool.tile([128, 1024], bf16, name="sq_scr", tag="sq_scr")
                nc.scalar.activation(
                    sq_scr[:], xf[t][:, 1024 * c4:1024 * (c4 + 1)],
                    Act.Square, accum_out=spart[t][:, 4 + c4:5 + c4])
        for t in range(T):
            nc.vector.tensor_tensor(
                spart[t][:, 0:1], spart[t][:, 0:1], spart[t][:, 1:2], Alu.add)
            nc.vector.tensor_tensor(
                spart[t][:, 2:3], spart[t][:, 2:3], spart[t][:, 3:4], Alu.add)
            nc.vector.tensor_tensor(
                stat[t][:, 0:1], spart[t][:, 0:1], spart[t][:, 2:3], Alu.add)
            nc.vector.tensor_tensor(
                spart[t][:, 4:5], spart[t][:, 4:5], spart[t][:, 5:6], Alu.add)
            nc.vector.tensor_tensor(
                spart[t][:, 6:7], spart[t][:, 6:7], spart[t][:, 7:8], Alu.add)
            nc.vector.tensor_tensor(
                stat[t][:, 1:2], spart[t][:, 4:5], spart[t][:, 6:7], Alu.add)
            # group-reduce over partitions via indicator matmuls:
            # gg[g,s] = sum_ch ind[ch,g]*stat[ch,s]; then broadcast back
            gg_ps = psum_a.tile([2, 2], f32, name="gg_ps", tag="sA")
            nc.tensor.matmul(gg_ps[0:2, :], ind[:, 0:2], stat[t][:, 0:2],
                             start=True, stop=True)
            gg_sb = left.tile([2, 2], f32, name=f"gg_sb{t}", tag=f"gg_sb{t}")
            nc.vector.tensor_copy(gg_sb[0:2, :], gg_ps[0:2, :])
            gb_ps = psum_b.tile([128, 2], f32, name="gb_ps", tag="sB")
            nc.tensor.matmul(gb_ps[:, 0:2], indT[0:2, :], gg_sb[0:2, :],
                             start=True, stop=True)
            nc.vector.tensor_copy(gstat[t][:, 0:2], gb_ps[:, 0:2])
            # mean/var/rstd -> per-channel affine a,b
            mean_t = left.tile([128, 1], f32, name=f"mean{t}", tag=f"mean{t}")
            e2_t = left.tile([128, 1], f32, name=f"e2{t}", tag=f"e2{t}")
            var_t = left.tile([128, 1], f32, name=f"var{t}", tag=f"var{t}")
            std_t = left.tile([128, 1], f32, name=f"std{t}", tag=f"std{t}")
            a_t = left.tile([128, 1], f32, name=f"a{t}", tag=f"a{t}")
            b_t = left.tile([128, 1], f32, name=f"b{t}", tag=f"b{t}")
            inv = 1.0 / GELEM
            nc.vector.tensor_scalar(mean_t[:], gstat[t][:, 0:1], inv, None, Alu.mult)
            nc.vector.tensor_scalar(e2_t[:], gstat[t][:, 1:2], inv, None, Alu.mult)
            nc.vector.scalar_tensor_tensor(
                var_t[:], mean_t[:], -1.0, mean_t[:], Alu.mult, Alu.mult)
            nc.vector.scalar_tensor_tensor(
                var_t[:], e2_t[:], EPS, var_t[:], Alu.add, Alu.add)
            nc.scalar.activation(std_t[:], var_t[:], Act.Sqrt)
            nc.vector.reciprocal(a_t[:], std_t[:])
            nc.vector.tensor_tensor(a_t[:], a_t[:], nwt[t][:], Alu.mult)
            nc.vector.tensor_tensor(b_t[:], mean_t[:], a_t[:], Alu.mult)
            nc.vector.tensor_tensor(b_t[:], nbt[t][:], b_t[:], Alu.subtract)
            # phase 2: normalize + cast
            nc.vector.tensor_scalar(
                xn[t][:], xf[t][:], a_t[:, 0:1], b_t[:, 0:1], Alu.mult, Alu.add)

        right_ctx.close()

        # ---------------- mid pools (reuse xf space) ----------------
        mid = ctx.enter_context(tc.tile_pool(name="mid", bufs=1))
        psb_pool = ctx.enter_context(tc.tile_pool(name="psb_pool", bufs=4))
        rec_pool = ctx.enter_context(tc.tile_pool(name="rec_pool", bufs=2))
        yo_pool = ctx.enter_context(tc.tile_pool(name="yo_pool", bufs=2))

        vT = mid.tile([128, MT * 520], bf16, name="vT", tag="vT")
        dup_pool = ctx.enter_context(tc.tile_pool(name="dup_pool", bufs=2))
        yh = [mid.tile([128, 512], f32, name=f"yh{i}", tag=f"yh{i}") for i in range(T)]
        attn = [mid.tile([128, NQ], bf16, name=f"attn{t}", tag=f"attn{t}") for t in range(T)]
        xres = [mid.tile([128, NQ], bf16, name=f"xres{t}", tag=f"xres{t}") for t in range(T)]
        for t in range(T):
            # gpsimd DMA casts f32->bf16 in flight (residual: 0.2% of |x|)
            nc.gpsimd.dma_start(out=xres[t][:], in_=xb[128 * t:128 * (t + 1), 0:NQ])

        # ones columns of the augmented v^T (denominator trick)
        ones_view = vT[:].rearrange("p (m h x) -> p m h x", m=MT, x=65)[:, :, :, 64:65]
        nc.vector.memset(ones_view, 1.0)

        # ---------------- phase 3: projections ----------------
        # q: out rows 0..511 of qkv, only window-0 columns here; window-1 q
        # and k tiles 1..3 are emitted later as attention-period fillers.
        def q_chain(i, w, par):
            qp = pool_ab[par].tile([128, 512], f32, name="qp",
                                   tag="sA" if par == 0 else "sB")
            for k in range(T):
                nc.tensor.matmul(
                    qp[:], wTq[k][:, 128 * i:128 * i + 128],
                    xn[k][:, 512 * w:512 * w + 512],
                    start=(k == 0), stop=(k == T - 1))
            nc.vector.tensor_scalar(
                qsb[i][:, 512 * w:512 * w + 512], qp[:], qb[i][:, 0:1], None, Alu.add)

        def k_chain(i, w, par):
            kp = pool_ab[par].tile([128, 512], f32, name="kp",
                                   tag="sA" if par == 0 else "sB")
            for k in range(T):
                nc.tensor.matmul(
                    kp[:], wTq[k][:, 512 + 128 * i:512 + 128 * i + 128],
                    xn[k][:, 512 * w:512 * w + 512],
                    start=(k == 0), stop=(k == T - 1))
            nc.vector.tensor_scalar(
                ksb[i][:, 512 * w:512 * w + 512], kp[:], qb[4 + i][:, 0:1], None, Alu.add)

        def v_chain(mt, par):
            vp = pool_ab[par].tile([128, 512], f32, name="vp",
                                   tag="sA" if par == 0 else "sB")
            for k in range(T):
                nc.tensor.matmul(
                    vp[:], xn[k][:, 128 * mt:128 * mt + 128],
                    wTq[k][:, 1024:1536],
                    start=(k == 0), stop=(k == T - 1))
            dst = vT[:, 520 * mt:520 * mt + 520].rearrange(
                "p (h x) -> p h x", x=65)[:, :, 0:64]
            srcv = vp[:].rearrange("p (h x) -> p h x", x=64)
            vbv = vb_bc[:].rearrange("p (h x) -> p h x", x=64)
            nc.vector.tensor_tensor(dst, srcv, vbv, Alu.add)

        def proj_chain(i, w, par, ks=(0, 1, 2, 3), partial=None, combine=None):
            py = pool_ab[par].tile([128, 512], f32, name="py",
                                   tag="sA" if par == 0 else "sB")
            for n_, k in enumerate(ks):
                nc.tensor.matmul(
                    py[:], wTp[k][:, 128 * i:128 * i + 128],
                    attn[k][:, 512 * w:512 * w + 512],
                    start=(n_ == 0), stop=(n_ == len(ks) - 1))
            if partial is not None:
                nc.vector.tensor_copy(partial[:], py[:])
                return
            yo = yo_pool.tile([128, 512], f32, name="yo", tag="yo")
            nc.vector.scalar_tensor_tensor(
                yo[:], py[:], pb[i][:, 0:1], xres[i][:, 512 * w:512 * w + 512],
                Alu.add, Alu.add)
            if combine is not None:
                nc.vector.tensor_tensor(yo[:], yo[:], combine[:], Alu.add)
            nc.sync.dma_start(y[128 * i:128 * i + 128, 512 * w:512 * w + 512], yo[:])

        # prefix: k tiles 0-1, window-0 q, all of vT (PV consumes vT from
        # the first attention period on)
        for w8 in range(8):
            k_chain(0, w8, w8 % 2)
            k_chain(1, w8, (w8 + 1) % 2)
        for i in range(T):
            q_chain(i, 0, i % 2)

        # ---------------- phase 4: attention (flat pipelined stream) ------
        # Global stream of periods over (window, pair, group). PV runs one
        # period behind S/exp; pair normalize is deferred into the next
        # pair's first period; filler chains (k tiles 1-3, window-1 q,
        # window-0 proj) are emitted on alternate periods.
        def gsize(r):
            return 3 if r < NGRP - 1 else MT - 3 * (NGRP - 1)

        periods = [(w, p, r) for w in range(W) for p in range(4)
                   for r in range(NGRP)]
        pair_state = {}
        dup_state = {}

        def prep_dup(w, p, half):
            # swapped-row copies: kdup/qdup rows 64-127 hold head h0's data,
            # rows 0-63 hold h1's, so alternate S matmuls can run on disjoint
            # PE row halves (row-tiling concurrency). Pure relocation - no
            # numerical change. SBUF->SBUF DMA on otherwise idle engines.
            # Split in column halves so each DMA is emitted strictly after
            # the filler chains producing its source columns.
            if half == 0:
                kd = dup_pool.tile([128, N], bf16, name="kdup", tag="kdup")
                qd = dup_pool.tile([128, NQ], bf16, name="qdup", tag="qdup")
                dup_state[(w, p)] = (kd, qd)
                # q: only this pair's window columns are ever read
                nc.sync.dma_start(qd[64:128, 512 * w:512 * w + 512],
                                  qsb[p][0:64, 512 * w:512 * w + 512])
                nc.sync.dma_start(qd[0:64, 512 * w:512 * w + 512],
                                  qsb[p][64:128, 512 * w:512 * w + 512])
                lo, hi = 0, 2048
            else:
                kd, qd = dup_state[(w, p)]
                lo, hi = 2048, N
            nc.sync.dma_start(kd[64:128, lo:hi], ksb[p][0:64, lo:hi])
            nc.sync.dma_start(kd[0:64, lo:hi], ksb[p][64:128, lo:hi])

        def emit_pv(w, p, r):
            pvs, ps_t = pair_state[(w, p)]
            if pvs[0] is None:
                for hh in range(2):
                    pvs[hh] = psum_pv.tile([128, 512], f32, name=f"pv{hh}", tag="pv")
            gs = gsize(r)
            for hh in range(2):
                h = 2 * p + hh
                pst = ps_t[hh][r]
                for j in range(gs):
                    m = 3 * r + j
                    nc.tensor.matmul(
                        pvs[hh][0:65, :],
                        vT[:, 520 * m + 65 * h:520 * m + 65 * h + 65],
                        pst[:, 512 * j:512 * j + 512],
                        start=(m == 0), stop=(m == MT - 1))

        def emit_normalize(w, p):
            pvs, _ = pair_state[(w, p)]
            for hh in range(2):
                h = 2 * p + hh
                kt, prr = h // 2, 64 * (h % 2)
                pvc = rec_pool.tile([65, 512], f32, name="pvc", tag="pvc")
                nc.vector.tensor_copy(pvc[0:65, :], pvs[hh][0:65, :])
                dnm = rec_pool.tile([1, 512], f32, name="dnm", tag="dnm")
                nc.vector.tensor_copy(dnm[0:1, :], pvc[64:65, :])
                rec = rec_pool.tile([1, 512], f32, name="rec", tag="rec")
                rscr = rec_pool.tile([1, 512], f32, name="rscr", tag="rscr")
                nc.vector.reciprocal_approx_accurate(
                    rec[0:1, :], dnm[0:1, :], rscr[0:1, :])
                bc = pool_ab[hh].tile([64, 512], f32, name="bc",
                                     tag="sA" if hh == 0 else "sB")
                nc.tensor.matmul(
                    bc[0:64, :], ones_row[0:1, 0:D],
                    rec[0:1, :], start=True, stop=True)
                bcs = rec_pool.tile([64, 512], f32, name="bcs", tag="bcs")
                nc.vector.tensor_copy(bcs[0:64, :], bc[0:64, :])
                nc.vector.tensor_tensor(
                    attn[kt][prr:prr + 64, 512 * w:512 * w + 512],
                    pvc[0:64, :], bcs[0:64, :], Alu.mult)

        # filler schedule: (earliest_period, closure); one pop on EVEN
        # periods, always from pool B (its next S alloc has ~2x more slack
        # than pool A's, so the filler's drain never delays the exp stream).
        # Emission deadlines: ksb[2] before period 22, ksb[3] before 33,
        # window-1 q before 44, window-0 proj after normalize(w0,p3) at 44.
        fillers = []
        for i in range(2, T):                       # ksb[2..3]: pops 2..16, 18..32
            for w8 in range(8):
                fillers.append((16 * (i - 2) + 2 + 2 * w8,
                                lambda i=i, w8=w8: k_chain(i, w8, 1)))
        for i in range(T):                          # window-1 q: pops 34..40
            fillers.append((34 + 2 * i, lambda i=i: q_chain(i, 1, 1)))
        for i in range(T):                          # window-0 proj: pops 46..52
            fillers.append((46 + 2 * i, lambda i=i: proj_chain(i, 0, 1)))
        for i in range(T):                          # w1 proj half (pairs 0-1)
            fillers.append((70 + 2 * i, lambda i=i:
                            proj_chain(i, 1, 1, ks=(0, 1), partial=yh[i])))
        fillers.reverse()   # pop from the end

        prep_dup(0, 0, 0)
        prep_dup(0, 0, 1)
        for mt in range(MT):
            v_chain(mt, mt % 2)
        for g, (w, p, r) in enumerate(periods):
            gs = gsize(r)
            if r == 0:
                pair_state[(w, p)] = (
                    [None, None], [[None] * NGRP, [None] * NGRP])
            pvs, ps_t = pair_state[(w, p)]
            kd, qd = dup_state[(w, p)]
            for hh in range(2):
                sp = pool_ab[hh].tile([128, 512 * gs], f32, name=f"sp{hh}",
                                      tag="sA" if hh == 0 else "sB")
                for j in range(gs):
                    m = 3 * r + j
                    # even j: head's native row half; odd j: the swapped copy
                    # on the opposite half -> adjacent matmuls use disjoint
                    # PE row groups and run concurrently
                    if j % 2 == 0:
                        pr, kt_, qt_ = 64 * hh, ksb[p], qsb[p]
                    else:
                        pr, kt_, qt_ = 64 * (1 - hh), kd, qd
                    nc.tensor.matmul(
                        sp[:, 512 * j:512 * j + 512],
                        kt_[pr:pr + 64, 128 * m:128 * m + 128],
                        qt_[pr:pr + 64, 512 * w:512 * w + 512],
                        start=True, stop=True)
                pst = psb_pool.tile([128, 1536], bf16, name="ps", tag="ps")
                nc.scalar.activation(
                    pst[:, 0:512 * gs], sp[:, 0:512 * gs], Act.Exp, scale=0.125)
                ps_t[hh][r] = pst
                # PV of the previous period goes between the two S blocks
                # (h0 part) and after them (h1 part)
                if hh == 0:
                    if g > 0:
                        pw, pp, prr_ = periods[g - 1]
                        emit_pv(pw, pp, prr_)
            if g > 0 and periods[g - 1][2] == NGRP - 1:
                emit_normalize(*periods[g - 1][:2])
            if g % 2 == 0 and fillers and fillers[-1][0] <= g:
                fillers.pop()[1]()
            if r == 5 and g + 6 < len(periods):
                prep_dup(*periods[g + 6][:2], 0)
            if r == NGRP - 1 and g + 1 < len(periods):
                prep_dup(*periods[g + 1][:2], 1)

        # tail: last period's PV, last normalize, window-1 proj second half
        emit_pv(*periods[-1])
        emit_normalize(W - 1, 3)
        while fillers:
            fillers.pop()[1]()
        for i in range(T):
            proj_chain(i, 1, i % 2, ks=(2, 3), combine=yh[i])


def _build():
    import concourse.tile as tile
    from concourse import bacc, mybir

    nc = bacc.Bacc("TRN2", target_bir_lowering=False, debug=False)
    f32 = mybir.dt.float32
    io = {
        "xb": nc.dram_tensor("xb", [C, N], f32, kind="ExternalInput").ap(),
        "qkvw": nc.dram_tensor("qkvw", [3 * C, C], f32, kind="ExternalInput").ap(),
        "qkvb": nc.dram_tensor("qkvb", [3 * C], f32, kind="ExternalInput").ap(),
        "projw": nc.dram_tensor("projw", [C, C], f32, kind="ExternalInput").ap(),
        "projb": nc.dram_tensor("projb", [C], f32, kind="ExternalInput").ap(),
        "nw": nc.dram_tensor("nw", [C], f32, kind="ExternalInput").ap(),
        "nb": nc.dram_tensor("nb", [C], f32, kind="ExternalInput").ap(),
        "cid": nc.dram_tensor("cid", [128, 128], mybir.dt.bfloat16,
                              kind="ExternalInput").ap(),
        "cind": nc.dram_tensor("cind", [128, 2], f32, kind="ExternalInput").ap(),
        "cindT": nc.dram_tensor("cindT", [2, 128], f32, kind="ExternalInput").ap(),
        "y": nc.dram_tensor("y", [C, NQ], f32, kind="ExternalOutput").ap(),
    }
    with tile.TileContext(nc) as tc:
        _emit(tc, io)
    nc.compile()
    return nc


def get_compiled():
    global _COMPILED
    if _COMPILED is None:
        _COMPILED = _build()
    return _COMPILED


def make_in_maps(x, norm_w, norm_b, qkv_w, qkv_b, proj_w, proj_b):
    import ml_dtypes

    xf = np.ascontiguousarray(np.asarray(x, np.float32)).reshape(2, C, N)
    ind = np.zeros((128, 2), np.float32)
    ind[0:64, 0] = 1.0
    ind[64:128, 1] = 1.0
    shared = {
        "cid": np.eye(128, dtype=ml_dtypes.bfloat16),
        "cind": ind,
        "cindT": np.ascontiguousarray(ind.T),
        "qkvw": np.ascontiguousarray(np.asarray(qkv_w, np.float32)),
        "qkvb": np.ascontiguousarray(np.asarray(qkv_b, np.float32)),
        "projw": np.ascontiguousarray(np.asarray(proj_w, np.float32)),
        "projb": np.ascontiguousarray(np.asarray(proj_b, np.float32)),
        "nw": np.ascontiguousarray(np.asarray(norm_w, np.float32)),
        "nb": np.ascontiguousarray(np.asarray(norm_b, np.float32)),
    }
    in_maps = []
    for core in range(8):
        bi, qs = core // 4, core % 4
        # rotate so this core's queries are always columns [0:NQ)
        xroll = np.concatenate(
            [xf[bi][:, qs * NQ:], xf[bi][:, :qs * NQ]], axis=1)
        m = dict(shared)
        m["xb"] = np.ascontiguousarray(xroll)
        in_maps.append(m)
    return in_maps


def assemble(results, x):
    y = np.zeros((2, C, N), np.float32)
    for core in range(8):
        bi, qs = core // 4, core % 4
        y[bi][:, qs * NQ:(qs + 1) * NQ] = results[core]["y"]
    return y.reshape(x.shape)


def kernel(x, norm_w, norm_b, qkv_w, qkv_b, proj_w, proj_b, **_ignored):
    from concourse import bass_utils

    nc = get_compiled()
    in_maps = make_in_maps(x, norm_w, norm_b, qkv_w, qkv_b, proj_w, proj_b)
    res = bass_utils.run_bass_kernel_spmd(nc, in_maps, core_ids=list(range(8)))
    return assemble(res.results, np.asarray(x))
